# revision 29
# baseline (speedup 1.0000x reference)
"""BinaryLinear (sign-binarized weight linear layer) on 8 Trainium2 NeuronCores.

y[b,s,o] = sum_i x[b,s,i] * (scale[o] * sign(w[o,i])) + bias[o]
  with scale[o] = mean_i |w[o,i]|

Sharding: data-parallel over the batch dim (8 batches -> 8 cores); w/bias
replicated. Per core (m = sequence rows, o = out features, k = in features):

  - x and w stream in as bf16 via SWDGE cast-DMA (DRAM f32 -> SBUF bf16),
    halving their HBM traffic; sign/scale tolerate the bf16 w rounding
  - x^T built by XBAR DMA-transpose from the SBUF slabs; w binarized by ACT
    Sign in SBUF, then B^T XBAR-transposed from SBUF (no DRAM round trips)
  - TensorE: yT[o,m] = B^T.T @ x^T accumulated over k in PSUM (bf16 inputs,
    f32 accumulate); DVE fuses psum*scale[o]+bias[o] on PSUM eviction,
    emitting bf16 (host upcasts; the 0.4% rounding is well inside tolerance)
  - ALL XBAR transposes issue on the sync queue ONLY: concurrent transposes
    on both HWDGE queues corrupt each other (measured), and the scheduler
    serializes a transpose against every other in-flight DMA regardless
  - graduated warm-up: early o-blocks run only the n-chunks already built,
    giving the x^T build DMA slack that an all-chunks start would not have;
    their missing n-chunks run at the end from a DRAM spill of B
"""

import numpy as np

B_DIM = 8
S_DIM = 2048
IN_F = 4096
OUT_F = 4096
P = 128
N_CORES = 8
N_TILE = 512

_BUILT = None


def _build_nc(s_dim=S_DIM, in_f=IN_F, out_f=OUT_F):
    from contextlib import ExitStack

    import concourse.mybir as mybir
    import concourse.tile as tile
    from concourse import bacc
    from concourse.bass import ts

    f32 = mybir.dt.float32
    bf16 = mybir.dt.bfloat16

    NCH = s_dim // N_TILE  # n chunks (moving-dim tiles of 512)
    PO = out_f // P  # o blocks (output-partition tiles of 128)
    KT = in_f // P  # contraction subtiles of 128
    NSLAB = s_dim // P  # x slabs (128 rows each)
    SL_CH = N_TILE // P  # slabs per chunk
    # graduated warm-up phases: (o_start, o_end, n_count).
    # ORDERING INVARIANT: the matmul's strided rhs read of x^T is NOT
    # dependency-tracked against the slab transposes (observed race), so
    # correctness relies on sync-queue FIFO: every slab transpose of chunk c
    # must be EMITTED before the bt transpose of the first block reading c
    # (the matmul's dep on its contiguous bt read is real, and the bt
    # transpose completes only after all earlier sync-queue transposes).
    # With 2 slabs emitted per block ahead of the bt, slabs available before
    # bt[m] = 2m+6; first readers below need 8/12/16 at m=1/3/7.
    if NCH > 1:
        GRAD = [(0, 1, 1), (1, 3, 2), (3, 7, 3), (7, PO, NCH)]
        CLEAN = [(0, 1, 1), (1, 3, 2), (3, 7, 3)]
    else:
        GRAD = [(0, PO, 1)]
        CLEAN = []

    nc = bacc.Bacc(None, target_bir_lowering=False, debug=False)
    with tile.TileContext(nc) as tc:
        x_d = nc.dram_tensor("x", (s_dim, in_f), f32, kind="ExternalInput")
        w_d = nc.dram_tensor("w", (out_f, in_f), f32, kind="ExternalInput")
        b_d = nc.dram_tensor("bias", (out_f,), f32, kind="ExternalInput")
        yT_d = nc.dram_tensor("yT", (out_f, s_dim), bf16, kind="ExternalOutput")

        with ExitStack() as ctx:
            yT3 = yT_d[:, :].rearrange("(po pi) s -> pi po s", pi=P)

            const = ctx.enter_context(tc.tile_pool(name="const", bufs=1))
            # slab-major layout: each XBAR transpose writes one fully
            # contiguous [P, KT, P] block (a strided per-slab footprint is
            # mis-modeled by the dependency tracker -> matmuls race the
            # transpose); the matmul reads across slabs with a 3D AP
            xT = const.tile([P, NSLAB, KT, P], bf16)  # resident x^T
            scale_sb = const.tile([P, PO], f32)
            bias_sb = const.tile([P, PO], f32)
            nc.scalar.dma_start(bias_sb[:], b_d[:].rearrange("(po pi) -> pi po", pi=P))

            wpool = ctx.enter_context(tc.tile_pool(name="wpool", bufs=2))
            bpool = ctx.enter_context(tc.tile_pool(name="bpool", bufs=2))
            xpool = ctx.enter_context(tc.tile_pool(name="xpool", bufs=3))
            btpool = ctx.enter_context(tc.tile_pool(name="btpool", bufs=2))
            opool = ctx.enter_context(tc.tile_pool(name="opool", bufs=7))
            psum = ctx.enter_context(tc.tile_pool(name="psum", bufs=6, space="PSUM"))

            # ---- x pipeline: SWDGE cast-load slab -> XBAR transpose ----
            x_tiles = {}
            next_load = 0

            def load_x_slab():
                nonlocal next_load
                if next_load >= NSLAB:
                    return
                g = next_load
                next_load += 1
                xr = xpool.tile([P, in_f], bf16, tag="xr", name=f"x_{g}")
                nc.gpsimd.dma_start(xr[:], x_d[ts(g, P), :])
                x_tiles[g] = xr

            next_slab = 0

            def build_x_slab():
                nonlocal next_slab
                if next_slab >= NSLAB:
                    return
                g = next_slab
                next_slab += 1
                nc.sync.dma_start_transpose(
                    xT[:, g, :, :],
                    x_tiles.pop(g)[:],
                )

            # ---- w pipeline: SWDGE cast-load -> ACT sign -> XBAR B^T ----
            w_tiles = {}

            def load_w(m, gen=0):
                w_sb = wpool.tile([P, in_f], bf16, tag="w", name=f"w_{m}_{gen}")
                nc.gpsimd.dma_start(w_sb[:], w_d[ts(m, P), :])
                w_tiles[m] = w_sb

            b_tiles = {}

            def process_w(m, with_scale=True):
                b_sb = bpool.tile([P, in_f], bf16)
                w_sb = w_tiles.pop(m)
                nc.scalar.sign(b_sb[:], w_sb[:])
                if with_scale:
                    nc.vector.tensor_reduce(
                        scale_sb[:, m : m + 1],
                        w_sb[:],
                        axis=mybir.AxisListType.X,
                        op=mybir.AluOpType.add,
                        apply_absolute_value=True,
                    )
                    nc.vector.tensor_scalar_mul(
                        scale_sb[:, m : m + 1], scale_sb[:, m : m + 1], 1.0 / in_f
                    )
                b_tiles[m] = b_sb

            def load_bt(m):
                bt = btpool.tile([P, KT, P], bf16)
                b3 = b_tiles.pop(m)[:].rearrange("o (kt ki) -> o kt ki", ki=P)
                nc.sync.dma_start_transpose(bt[:], b3)
                return bt

            def mm_block(bt, m, n):
                ps = psum.tile([P, N_TILE], f32, name="ps")
                for kt in range(KT):
                    nc.tensor.matmul(
                        ps[:],
                        bt[:, kt, :],
                        xT[:, ts(n, SL_CH), kt, :],
                        start=(kt == 0),
                        stop=(kt == KT - 1),
                    )
                ob = opool.tile([P, N_TILE], bf16)
                nc.vector.tensor_scalar(
                    ob[:],
                    ps[:],
                    scale_sb[:, m : m + 1],
                    bias_sb[:, m : m + 1],
                    op0=mybir.AluOpType.mult,
                    op1=mybir.AluOpType.add,
                )
                nc.scalar.dma_start(yT3[:, m, ts(n, N_TILE)], ob[:])

            # ---- emission ----
            # bootstrap: w0/w1 + first six x-slab loads, then the chunk-0
            # transposes
            load_w(0)
            load_w(1)
            for _ in range(SL_CH + 2):
                load_x_slab()
            process_w(0)
            next_proc = 1

            def advance_prep():
                nonlocal next_proc
                if next_proc < PO:
                    if next_proc + 1 < PO:
                        load_w(next_proc + 1)
                    process_w(next_proc)
                    next_proc += 1

            for _ in range(SL_CH):
                build_x_slab()

            for o0, o1, nct in GRAD:
                for m in range(o0, o1):
                    # slab loads + transposes BEFORE the bt transpose: the
                    # FIFO ordering invariant above depends on this
                    load_x_slab()
                    load_x_slab()
                    build_x_slab()
                    build_x_slab()
                    bt = load_bt(m)
                    advance_prep()
                    for n in range(nct):
                        mm_block(bt, m, n)
            # cleanup: the n-chunks the warm-up skipped; w is cheap to
            # re-load (bf16 cast) and re-sign, so no spill round trip
            for o0, o1, nct in CLEAN:
                for m in range(o0, o1):
                    load_w(m, gen=1)
            for o0, o1, nct in CLEAN:
                for m in range(o0, o1):
                    process_w(m, with_scale=False)
                    bt = load_bt(m)
                    for n in range(nct, NCH):
                        mm_block(bt, m, n)
    nc.finalize()
    return nc


def _get_nc():
    global _BUILT
    if _BUILT is None:
        _BUILT = _build_nc()
    return _BUILT


def kernel(x, weight, bias):
    from concourse.bass_utils import run_bass_kernel_spmd

    x = np.asarray(x, dtype=np.float32)
    weight = np.asarray(weight, dtype=np.float32)
    bias = np.asarray(bias, dtype=np.float32)
    assert x.shape == (B_DIM, S_DIM, IN_F), x.shape

    nc = _get_nc()
    in_maps = [
        {"x": np.ascontiguousarray(x[b]), "w": weight, "bias": bias}
        for b in range(N_CORES)
    ]
    res = run_bass_kernel_spmd(nc, in_maps, core_ids=list(range(N_CORES)))
    out = np.empty((B_DIM, S_DIM, OUT_F), dtype=np.float32)
    for b in range(N_CORES):
        out[b] = res.results[b]["yT"].astype(np.float32).T
    return out
